# revision 1
# baseline (speedup 1.0000x reference)
"""GroupViT cross-attention layer on 8 TRN2 NeuronCores.

Strategy: pure data-parallel over batch (16 batches -> 2 per core, zero
collectives). Feature-major ("transposed") layout on chip: activations
stored [feature(partition), token(free)], weights host-transposed to
[d_in, d_out] so every matmul contracts over the partition dim.

dtypes: attention path bf16 (its output is ~1% of the residual stream,
errors diluted ~86x), MLP/residual/LN path float32r (~1e-4 matmul error
at full PE speed for free-dim >= 256).

Softmax: scores are O(+-3) so exp needs no max subtraction. Scores are
computed transposed [s, t]; denominators come free from a ones column
appended to V in the ctx matmul; normalization via a k=1 outer-product
broadcast matmul + one DVE multiply per head.

LN over the partition (feature) dim: sums via ones-column matmuls,
(x - mu)*rs*g + b applied as x (*) (g (x) rs) - (g (x) mu*rs - b (x) 1)
with the broadcast tensors built by tiny k=1 matmuls into PSUM.
"""

import numpy as np

B, T, S, D, H, HD, FF = 16, 512, 2048, 768, 12, 64, 3072
NCORES = 8
BPC = B // NCORES      # batches per core
P = 128
DC = D // P            # 6 feature chunks
SC = S // P            # 16 key-token chunks
FFC = FF // P          # 24
EPS = 1e-5
SCALE = HD ** -0.5

_cached = {}


def _build(use_bv: bool):
    import concourse.bacc as bacc
    import concourse.tile as tile
    import concourse.mybir as mybir

    f32 = mybir.dt.float32
    f32r = mybir.dt.float32r
    bf16 = mybir.dt.bfloat16
    AF = mybir.ActivationFunctionType
    ALU = mybir.AluOpType

    nc = bacc.Bacc("TRN2", target_bir_lowering=False, debug=False,
                   num_devices=NCORES)

    # ---- DRAM I/O (per-core shapes) ----
    qT_d = nc.dram_tensor("qT", [BPC, D, T], f32r, kind="ExternalInput")
    kT_d = nc.dram_tensor("kT", [BPC, D, S], f32, kind="ExternalInput")
    wq_d = nc.dram_tensor("wq_t", [D, D], f32r, kind="ExternalInput")
    wk_d = nc.dram_tensor("wk_t", [D, D], f32, kind="ExternalInput")
    wv_d = nc.dram_tensor("wv_t", [D, D], f32, kind="ExternalInput")
    wo_d = nc.dram_tensor("wo_t", [D, D], f32, kind="ExternalInput")
    fc1_d = nc.dram_tensor("fc1_t", [D, FF], f32r, kind="ExternalInput")
    fc2_d = nc.dram_tensor("fc2_t", [FF, D], f32r, kind="ExternalInput")
    bq_d = nc.dram_tensor("bqv", [D], f32, kind="ExternalInput")
    bk_d = nc.dram_tensor("bkv", [D], f32, kind="ExternalInput")
    bv_d = nc.dram_tensor("bvv", [1, D], f32r, kind="ExternalInput")
    bo_d = nc.dram_tensor("bov", [D], f32, kind="ExternalInput")
    f1b_d = nc.dram_tensor("f1b", [FF], f32, kind="ExternalInput")
    f2b_d = nc.dram_tensor("f2b", [D], f32, kind="ExternalInput")
    ln2g_d = nc.dram_tensor("ln2g", [1, D], f32r, kind="ExternalInput")
    ln2bn_d = nc.dram_tensor("ln2bn", [1, D], f32r, kind="ExternalInput")
    lnpg_d = nc.dram_tensor("lnpg", [1, D], f32r, kind="ExternalInput")
    lnpbn_d = nc.dram_tensor("lnpbn", [1, D], f32r, kind="ExternalInput")
    ones_col_d = nc.dram_tensor("ones_col", [P, 1], f32r, kind="ExternalInput")
    ones_row_d = nc.dram_tensor("ones_row", [1, T], f32r, kind="ExternalInput")
    out_d = nc.dram_tensor("out", [BPC, D, T], f32, kind="ExternalOutput")

    def F(ap):
        return ap.bitcast(f32)

    def act_reciprocal(out, in_):
        eng = nc.scalar
        ins = [eng.lower_ap(in_)]
        for v in (0.0, 1.0, 0.0):
            ins.append(mybir.ImmediateValue(dtype=f32, value=v))
        return eng.add_instruction(mybir.InstActivation(
            name=nc.get_next_instruction_name(),
            func=AF.Reciprocal, ins=ins, outs=[eng.lower_ap(out)]))

    with tile.TileContext(nc) as tc:
        with (
            tc.tile_pool(name="act", bufs=3) as act,
            tc.tile_pool(name="bigk", bufs=1) as bigk,
            tc.tile_pool(name="vpool", bufs=1) as vpool,
            tc.tile_pool(name="qtp", bufs=1) as qtp,
            tc.tile_pool(name="ktc", bufs=2) as ktc,
            tc.tile_pool(name="wstream", bufs=2) as wstream,
            tc.tile_pool(name="wvp", bufs=1) as wvp,
            tc.tile_pool(name="fstream", bufs=2) as fstream,
            tc.tile_pool(name="expp", bufs=3) as expp,
            tc.tile_pool(name="mchunk", bufs=3) as mchunkp,
            tc.tile_pool(name="tmp", bufs=3) as tmpp,
            tc.tile_pool(name="small", bufs=1) as small,
        ):
            # ---- persistent small tiles ----
            ones_col = small.tile([P, 1], f32r, tag="ones_col")
            nc.sync.dma_start(ones_col[:], ones_col_d.ap())
            ones_row = small.tile([1, T], f32r, tag="ones_row")
            nc.sync.dma_start(ones_row[:], ones_row_d.ap())
            ones64_f = small.tile([1, HD], f32, tag="ones64f")
            nc.vector.memset(ones64_f[:], 1.0)
            ones64_bf = small.tile([1, HD], bf16, tag="ones64")
            nc.vector.tensor_copy(ones64_bf[:], ones64_f[:])
            onesc_f = small.tile([P, 1], f32, tag="onesc_f")
            nc.vector.memset(onesc_f[:], 1.0)
            eps_t = small.tile([1, 1], f32, tag="eps")
            nc.vector.memset(eps_t[:], EPS)

            ln2g = small.tile([1, D], f32r, tag="ln2g")
            nc.sync.dma_start(ln2g[:], ln2g_d.ap())
            ln2bn = small.tile([1, D], f32r, tag="ln2bn")
            nc.sync.dma_start(ln2bn[:], ln2bn_d.ap())
            lnpg = small.tile([1, D], f32r, tag="lnpg")
            nc.sync.dma_start(lnpg[:], lnpg_d.ap())
            lnpbn = small.tile([1, D], f32r, tag="lnpbn")
            nc.sync.dma_start(lnpbn[:], lnpbn_d.ap())

            bq_pc = small.tile([P, DC], f32, tag="bq_pc")
            nc.sync.dma_start(bq_pc[:], bq_d.ap().rearrange("(c p) -> p c", p=P))
            bk_pc = small.tile([P, DC], f32, tag="bk_pc")
            nc.sync.dma_start(bk_pc[:], bk_d.ap().rearrange("(c p) -> p c", p=P))
            bo_pc = small.tile([P, DC], f32, tag="bo_pc")
            nc.sync.dma_start(bo_pc[:], bo_d.ap().rearrange("(c p) -> p c", p=P))
            f1b_pc = small.tile([P, FFC], f32, tag="f1b_pc")
            nc.sync.dma_start(f1b_pc[:], f1b_d.ap().rearrange("(c p) -> p c", p=P))
            f2b_pc = small.tile([P, DC], f32, tag="f2b_pc")
            nc.sync.dma_start(f2b_pc[:], f2b_d.ap().rearrange("(c p) -> p c", p=P))

            bv_row = None
            if use_bv:
                bv_row = small.tile([1, D], f32r, tag="bv_row")
                nc.sync.dma_start(bv_row[:], bv_d.ap())

            def ln_pass(xsrc, dst, g_row, bn_row, ps_scope):
                """LayerNorm over the partition(feature) dim:
                xsrc [P, DC, T] f32r -> dst [P, DC, T]."""
                ps_st, ps_bc = ps_scope
                psum_mu = ps_st.tile([1, T], f32, tag="st_mu")
                psum_sq = ps_st.tile([1, T], f32, tag="st_sq")
                for c in range(DC):
                    nc.tensor.matmul(psum_mu[:], ones_col[:], xsrc[:, c, :],
                                     start=(c == 0), stop=(c == DC - 1))
                sqt = []
                for c in range(DC):
                    sq = tmpp.tile([P, T], f32r, tag="lnsq")
                    nc.vector.tensor_mul(sq[:], F(xsrc[:, c, :]),
                                         F(xsrc[:, c, :]))
                    sqt.append(sq)
                for c in range(DC):
                    nc.tensor.matmul(psum_sq[:], ones_col[:], sqt[c][:],
                                     start=(c == 0), stop=(c == DC - 1))
                mu_f = small.tile([1, T], f32, tag="ln_mu")
                nc.vector.tensor_scalar_mul(mu_f[:], psum_mu[:], 1.0 / D)
                mu2_f = small.tile([1, T], f32, tag="ln_mu2")
                nc.vector.tensor_tensor(mu2_f[:], mu_f[:], mu_f[:], ALU.mult)
                var_f = small.tile([1, T], f32, tag="ln_var")
                nc.vector.scalar_tensor_tensor(
                    var_f[:], psum_sq[:], 1.0 / D, mu2_f[:],
                    op0=ALU.mult, op1=ALU.subtract)
                rs_f = small.tile([1, T], f32, tag="ln_rs")
                nc.scalar.activation(rs_f[:], var_f[:], AF.Abs_reciprocal_sqrt,
                                     bias=eps_t[:])
                rs_r = small.tile([1, T], f32r, tag="ln_rs_r")
                nc.vector.tensor_copy(rs_r[:], rs_f[:])
                mrs_r = small.tile([1, T], f32r, tag="ln_mrs_r")
                nc.vector.tensor_tensor(mrs_r[:], mu_f[:], rs_f[:], ALU.mult)
                for c in range(DC):
                    bcA = ps_bc.tile([P, T], f32, tag="ln_bcA")
                    bcB = ps_bc.tile([P, T], f32, tag="ln_bcB")
                    gsl = g_row[:, c * P:(c + 1) * P]
                    bsl = bn_row[:, c * P:(c + 1) * P]
                    nc.tensor.matmul(bcA[:], gsl, rs_r[:], start=True, stop=True)
                    nc.tensor.matmul(bcB[:], gsl, mrs_r[:], start=True, stop=False)
                    nc.tensor.matmul(bcB[:], bsl, ones_row[:], start=False, stop=True)
                    tmp = tmpp.tile([P, T], f32, tag="ln_tmp")
                    nc.vector.tensor_tensor(tmp[:], F(xsrc[:, c, :]), bcA[:],
                                            ALU.mult)
                    nc.vector.tensor_tensor(dst[:, c, :], tmp[:], bcB[:],
                                            ALU.subtract)

            for b in range(BPC):
                # ================= phase A: load + Q/V projections ======
                qin = act.tile([P, DC, T], f32r, tag="act")
                nc.sync.dma_start(qin[:], qT_d.ap()[b].rearrange(
                    "(c p) t -> p c t", p=P))
                kin = bigk.tile([P, DC, S], bf16, tag="kin")
                nc.gpsimd.dma_start(kin[:], kT_d.ap()[b].rearrange(
                    "(c p) s -> p c s", p=P))
                wv_sb = wvp.tile([P, DC, D], bf16, tag="wv")
                nc.gpsimd.dma_start(wv_sb[:], wv_d.ap().rearrange(
                    "(k p) o -> p k o", p=P))

                qt = qtp.tile([P, DC, T], bf16, tag="qt")
                with tc.tile_pool(name="psA", bufs=2, space="PSUM") as psA:
                    for mo in range(DC):
                        wq_sl = wstream.tile([P, DC, P], f32r, tag="wq_sl")
                        nc.sync.dma_start(wq_sl[:], wq_d.ap().rearrange(
                            "(k p) o -> p k o", p=P)[:, :, mo * P:(mo + 1) * P])
                        ps = psA.tile([P, T], f32, tag="psA")
                        for ki in range(DC):
                            nc.tensor.matmul(ps[:], wq_sl[:, ki, :],
                                             qin[:, ki, :],
                                             start=(ki == 0), stop=(ki == DC - 1))
                        nc.vector.tensor_scalar_add(qt[:, mo, :], ps[:],
                                                    bq_pc[:, mo:mo + 1])

                    v_sb = vpool.tile([P, SC, H, HD + 1], bf16, tag="v")
                    nc.vector.tensor_copy(
                        v_sb[:, :, :, HD:HD + 1],
                        onesc_f[:].to_broadcast([P, SC, H, 1]))
                    bv_bc = None
                    if use_bv:
                        bv_bc = small.tile([P, D], f32, tag="bv_bc")
                        for half in range(2):
                            ps_bv = psA.tile([P, 384], f32, tag="psA")
                            nc.tensor.matmul(
                                ps_bv[:], ones_row[:, 0:P],
                                bv_row[:, half * 384:(half + 1) * 384],
                                start=True, stop=True)
                            nc.vector.tensor_copy(
                                bv_bc[:, half * 384:(half + 1) * 384], ps_bv[:])
                    for so in range(SC):
                        for half in range(2):
                            ps = psA.tile([P, 384], f32, tag="psA")
                            for ki in range(DC):
                                nc.tensor.matmul(
                                    ps[:],
                                    kin[:, ki, so * P:(so + 1) * P],
                                    wv_sb[:, ki, half * 384:(half + 1) * 384],
                                    start=(ki == 0), stop=(ki == DC - 1))
                            dstv = v_sb[:, so, half * 6:(half + 1) * 6, 0:HD]
                            if use_bv:
                                nc.vector.tensor_tensor(
                                    dstv, ps[:],
                                    bv_bc[:, half * 384:(half + 1) * 384],
                                    ALU.add)
                            else:
                                nc.vector.tensor_copy(dstv, ps[:])

                # ================= phase B: attention ====================
                ctxT = act.tile([P, DC, T], bf16, tag="act")

                def attn_kproj(hp, kin, psK):
                    wk_sl = wstream.tile([P, DC, P], bf16, tag="wk_sl")
                    nc.gpsimd.dma_start(wk_sl[:], wk_d.ap().rearrange(
                        "(k p) o -> p k o", p=P)[:, :, hp * P:(hp + 1) * P])
                    ktch = ktc.tile([P, S], bf16, tag="ktc")
                    for no in range(4):
                        ps = psK.tile([P, T], f32, tag="psK")
                        for ki in range(DC):
                            nc.tensor.matmul(
                                ps[:], wk_sl[:, ki, :],
                                kin[:, ki, no * T:(no + 1) * T],
                                start=(ki == 0), stop=(ki == DC - 1))
                        nc.vector.tensor_scalar_add(
                            ktch[:, no * T:(no + 1) * T], ps[:],
                            bk_pc[:, hp:hp + 1])
                    return ktch

                def attn_scores_ctx(hp, so2, ktch, qt, v_sb, ps_ctx, psSC):
                    scs = []
                    for hh in range(2):
                        base = hh * HD
                        ps_sc = psSC.tile([P, 2 * T], f32, tag="psSC",
                                          name=f"ps_sc{hh}")
                        for j in range(2):
                            so = so2 + j
                            nc.tensor.matmul(
                                ps_sc[:, j * T:(j + 1) * T],
                                ktch[base:base + HD, so * P:(so + 1) * P],
                                qt[base:base + HD, hp, :],
                                start=True, stop=True)
                        scs.append(ps_sc)
                    exs = []
                    for hh in range(2):
                        ex = expp.tile([P, 2 * T], bf16, tag="exp",
                                       name=f"ex{hh}")
                        nc.scalar.activation(ex[:], scs[hh][:], AF.Exp)
                        exs.append(ex)
                    for hh in range(2):
                        h = 2 * hp + hh
                        for j in range(2):
                            so = so2 + j
                            nc.tensor.matmul(
                                ps_ctx[hh][:], v_sb[:, so, h, :],
                                exs[hh][:, j * T:(j + 1) * T],
                                start=(so == 0), stop=(so == SC - 1))

                def attn_evict(hp, hh, ps_ctx, ctxT, psBC):
                    base = hh * HD
                    rden_f = tmpp.tile([1, T], f32, tag="rden_f")
                    act_reciprocal(rden_f[:], ps_ctx[hh][HD:HD + 1, :])
                    rden_bf = tmpp.tile([1, T], bf16, tag="rden_bf")
                    nc.vector.tensor_copy(rden_bf[:], rden_f[:])
                    ps_bc = psBC.tile([HD, T], f32, tag="psBC")
                    nc.tensor.matmul(ps_bc[:], ones64_bf[:],
                                     rden_bf[:], start=True, stop=True)
                    bc_sb = tmpp.tile([HD, T], f32, tag="bc_sb")
                    nc.vector.tensor_copy(bc_sb[:], ps_bc[:])
                    nc.vector.tensor_tensor(
                        ctxT[base:base + HD, hp, :],
                        ps_ctx[hh][0:HD, :], bc_sb[:], ALU.mult)

                with (
                    tc.tile_pool(name="psK", bufs=1, space="PSUM") as psK,
                    tc.tile_pool(name="psSC", bufs=2, space="PSUM") as psSC,
                    tc.tile_pool(name="psCTX", bufs=2, space="PSUM") as psCTX,
                    tc.tile_pool(name="psBC", bufs=1, space="PSUM") as psBC,
                ):
                    for hp in range(DC):
                        ktch = attn_kproj(hp, kin, psK)
                        ps_ctx = [psCTX.tile([HD + 1, T], f32, tag="psCTX",
                                            name=f"ps_ctx{i}")
                                  for i in range(2)]
                        for so2 in range(0, SC, 2):
                            attn_scores_ctx(hp, so2, ktch, qt, v_sb,
                                            ps_ctx, psSC)
                        for hh in range(2):
                            attn_evict(hp, hh, ps_ctx, ctxT, psBC)

                # ================= phase C: out_proj + residual ==========
                xT = act.tile([P, DC, T], f32r, tag="act")
                with tc.tile_pool(name="psC", bufs=2, space="PSUM") as psC:
                    for mo in range(DC):
                        wo_sl = wstream.tile([P, DC, P], bf16, tag="wo_sl")
                        nc.gpsimd.dma_start(wo_sl[:], wo_d.ap().rearrange(
                            "(k p) o -> p k o", p=P)[:, :, mo * P:(mo + 1) * P])
                        ps = psC.tile([P, T], f32, tag="psC")
                        for ki in range(DC):
                            nc.tensor.matmul(ps[:], wo_sl[:, ki, :],
                                             ctxT[:, ki, :],
                                             start=(ki == 0), stop=(ki == DC - 1))
                        nc.vector.scalar_tensor_tensor(
                            xT[:, mo, :], ps[:], bo_pc[:, mo:mo + 1],
                            F(qin[:, mo, :]), op0=ALU.add, op1=ALU.add)

                # ================= phase D: LN2 ==========================
                hT = act.tile([P, DC, T], f32r, tag="act")
                with (
                    tc.tile_pool(name="psST", bufs=1, space="PSUM") as psST,
                    tc.tile_pool(name="psLB", bufs=2, space="PSUM") as psLB,
                ):
                    ln_pass(xT, hT, ln2g, ln2bn, (psST, psLB))

                # ================= phase E: MLP (fused fc1->gelu->fc2) ===
                x2T = act.tile([P, DC, T], f32r, tag="act")
                with (
                    tc.tile_pool(name="psF1", bufs=2, space="PSUM") as psF1,
                    tc.tile_pool(name="psF2", bufs=6, space="PSUM") as psF2,
                ):
                    ps_f2 = [psF2.tile([P, T], f32, tag="psF2", name=f"ps_f2_{i}")
                             for i in range(DC)]
                    for fo in range(FFC):
                        f1_sl = fstream.tile([P, DC, P], f32r, tag="f1_sl")
                        nc.sync.dma_start(f1_sl[:], fc1_d.ap().rearrange(
                            "(k p) f -> p k f", p=P)[:, :, fo * P:(fo + 1) * P])
                        f2_sl = fstream.tile([P, D], f32r, tag="f2_sl")
                        nc.sync.dma_start(f2_sl[:], fc2_d.ap().rearrange(
                            "(ko p) o -> p ko o", p=P)[:, fo, :])
                        ps1 = psF1.tile([P, T], f32, tag="psF1")
                        for ki in range(DC):
                            nc.tensor.matmul(ps1[:], f1_sl[:, ki, :],
                                             hT[:, ki, :],
                                             start=(ki == 0), stop=(ki == DC - 1))
                        mch = mchunkp.tile([P, T], f32r, tag="mch")
                        nc.scalar.activation(mch[:], ps1[:], AF.Gelu,
                                             bias=f1b_pc[:, fo:fo + 1])
                        for mo in range(DC):
                            nc.tensor.matmul(
                                ps_f2[mo][:], f2_sl[:, mo * P:(mo + 1) * P],
                                mch[:],
                                start=(fo == 0), stop=(fo == FFC - 1))
                    for mo in range(DC):
                        nc.vector.scalar_tensor_tensor(
                            x2T[:, mo, :], ps_f2[mo][:], f2b_pc[:, mo:mo + 1],
                            F(xT[:, mo, :]), op0=ALU.add, op1=ALU.add)

                # ================= phase F: LNp + store ==================
                outT = act.tile([P, DC, T], f32, tag="act")
                with (
                    tc.tile_pool(name="psST2", bufs=1, space="PSUM") as psST2,
                    tc.tile_pool(name="psLB2", bufs=2, space="PSUM") as psLB2,
                ):
                    ln_pass(x2T, outT, lnpg, lnpbn, (psST2, psLB2))
                nc.sync.dma_start(
                    out_d.ap()[b].rearrange("(c p) t -> p c t", p=P), outT[:])

    nc.compile()
    return nc


def _get_nc(use_bv: bool):
    key = ("nc", use_bv)
    if key not in _cached:
        _cached[key] = _build(use_bv)
    return _cached[key]


def kernel(query, key, wq, bq, wk, bk, wv, bv, wo, bo,
           ln2_g, ln2_b, fc1_w, fc1_b, fc2_w, fc2_b, lnp_g, lnp_b):
    from concourse.bass_utils import run_bass_kernel_spmd

    f = np.float32
    c = np.ascontiguousarray
    query = np.asarray(query, f)
    key = np.asarray(key, f)
    use_bv = bool(np.any(np.asarray(bv)))
    nc = _get_nc(use_bv)

    shared = {
        "wq_t": c(np.asarray(wq, f).T * np.float32(SCALE)),
        "wk_t": c(np.asarray(wk, f).T),
        "wv_t": c(np.asarray(wv, f).T),
        "wo_t": c(np.asarray(wo, f).T),
        "fc1_t": c(np.asarray(fc1_w, f).T),
        "fc2_t": c(np.asarray(fc2_w, f).T),
        "bqv": c(np.asarray(bq, f) * np.float32(SCALE)),
        "bkv": c(np.asarray(bk, f)),
        "bvv": c(np.asarray(bv, f).reshape(1, D)),
        "bov": c(np.asarray(bo, f)),
        "f1b": c(np.asarray(fc1_b, f)),
        "f2b": c(np.asarray(fc2_b, f)),
        "ln2g": c(np.asarray(ln2_g, f).reshape(1, D)),
        "ln2bn": c(-np.asarray(ln2_b, f).reshape(1, D)),
        "lnpg": c(np.asarray(lnp_g, f).reshape(1, D)),
        "lnpbn": c(-np.asarray(lnp_b, f).reshape(1, D)),
        "ones_col": np.ones((P, 1), f),
        "ones_row": np.ones((1, T), f),
    }
    in_maps = []
    for core in range(NCORES):
        sl = slice(core * BPC, (core + 1) * BPC)
        m = dict(shared)
        m["qT"] = c(query[sl].transpose(0, 2, 1))
        m["kT"] = c(key[sl].transpose(0, 2, 1))
        in_maps.append(m)

    res = run_bass_kernel_spmd(nc, in_maps, core_ids=list(range(NCORES)))
    kernel._last_result = res
    out = np.concatenate([r["out"] for r in res.results], axis=0)
    return c(out.transpose(0, 2, 1))



# revision 19
# speedup vs baseline: 1.1966x; 1.1966x over previous
"""GroupViT cross-attention layer on 8 TRN2 NeuronCores.

Data-parallel over batch (16 batches -> 2 per core, zero collectives).
Feature-major layout on chip: activations [feature(partition), token(free)],
weights host-transposed so every matmul contracts over the partition dim.

v2: fp8(e4m3) DoubleRow matmuls for the Q/K/V/out projections and the
ctx (probs @ V) matmul -- DoubleRow contracts 256 rows per instruction,
halving PE instruction count vs bf16.  Scores stay bf16 (K=64 per head
cannot exploit DoubleRow).  MLP bf16.  Host converts query/key/weights
to fp8/bf16 so no on-chip casts of the big operands are needed.

Phase-serial schedule: [attn b0, attn b1] under the exp ACT table, then
[LN2 b0,b1][MLP b0,b1][LNp b0,b1] under rsqrt/gelu tables -- 4 ACT
table loads total instead of ~30.  Softmax reciprocal runs on DVE
(reciprocal_approx_fast); its broadcast across 64 rows is a k=1 matmul
into the unused upper partitions of the same ctx PSUM bank.
"""

import numpy as np
import ml_dtypes

B, T, S, D, H, HD, FF = 16, 512, 2048, 768, 12, 64, 3072
NCORES = 8
BPC = B // NCORES
P = 128
DC = D // P            # 6 feature chunks
KP = DC // 2           # 3 doublerow k-pairs
SC = S // P            # 16 key-token chunks
SP = SC // 2           # 8 key-token chunk pairs
FFC = FF // P          # 24
EPS = 1e-5
SCALE = HD ** -0.5

MLP_FP8 = False        # fc1/fc2 in fp8 DoubleRow

_cached = {}


def _build(use_bv: bool, mlp_fp8: bool):
    import concourse.bacc as bacc
    import concourse.tile as tile
    import concourse.mybir as mybir

    f32 = mybir.dt.float32
    f32r = mybir.dt.float32r
    bf16 = mybir.dt.bfloat16
    fp8 = mybir.dt.float8e4
    AF = mybir.ActivationFunctionType
    ALU = mybir.AluOpType
    DR = mybir.MatmulPerfMode.DoubleRow

    nc = bacc.Bacc("TRN2", target_bir_lowering=False, debug=False,
                   num_devices=NCORES)

    # ---- DRAM I/O (per-core shapes, host pre-tiled) ----
    qT8_d = nc.dram_tensor("qT8", [BPC, P, KP, 2, T], fp8, kind="ExternalInput")
    qTf_d = nc.dram_tensor("qTf", [BPC, P, DC, T], f32r, kind="ExternalInput")
    kT8_d = nc.dram_tensor("kT8", [BPC, P, KP, 2, S], fp8, kind="ExternalInput")
    w8q_d = nc.dram_tensor("w8q", [P, KP, 2, D], fp8, kind="ExternalInput")
    w8k_d = nc.dram_tensor("w8k", [P, KP, 2, D], fp8, kind="ExternalInput")
    w8v_d = nc.dram_tensor("w8v", [P, KP, 2, D], fp8, kind="ExternalInput")
    w8o_d = nc.dram_tensor("w8o", [P, KP, 2, D], fp8, kind="ExternalInput")
    if mlp_fp8:
        fc1_d = nc.dram_tensor("fc1_t", [FFC, P, KP, 2, P], fp8,
                               kind="ExternalInput")
        fc2_d = nc.dram_tensor("fc2_t", [FFC // 2, P, 2, D], fp8,
                               kind="ExternalInput")
    else:
        fc1_d = nc.dram_tensor("fc1_t", [FFC, P, DC, P], bf16,
                               kind="ExternalInput")
        fc2_d = nc.dram_tensor("fc2_t", [FFC, P, D], bf16,
                               kind="ExternalInput")
    bq_d = nc.dram_tensor("bqv", [P, DC], f32, kind="ExternalInput")
    bk_d = nc.dram_tensor("bkv", [P, DC], f32, kind="ExternalInput")
    bo_d = nc.dram_tensor("bov", [P, DC], f32, kind="ExternalInput")
    bv_d = nc.dram_tensor("bvv", [1, D], f32r, kind="ExternalInput")
    f1b_d = nc.dram_tensor("f1b", [P, FFC], f32, kind="ExternalInput")
    f2b_d = nc.dram_tensor("f2b", [P, DC], f32, kind="ExternalInput")
    # LN rows packed [4, D] bf16: 0=ln2g 1=-ln2b 2=lnpg 3=-lnpb
    lnrows_d = nc.dram_tensor("lnrows", [4, D], bf16, kind="ExternalInput")
    ones_col_d = nc.dram_tensor("ones_col", [P, 1], f32r, kind="ExternalInput")
    ones_row_d = nc.dram_tensor("ones_row", [1, T], f32r, kind="ExternalInput")
    out_d = nc.dram_tensor("out", [BPC, P, DC, T], f32, kind="ExternalOutput")

    def F(ap):
        return ap.bitcast(f32)

    from contextlib import ExitStack

    with tile.TileContext(nc) as tc:
        with ExitStack() as stack:
            pool_specs = [
                ("small", 1), ("wres", 1), ("kinp", 1), ("qinp", 2),
                ("qfp", 2), ("qtp", 2), ("ktc", 2), ("vp", 2), ("expp", 3),
                ("ctxp", 2), ("xp", 2), ("hp", 2), ("x2p", 2), ("outp", 2),
                ("sqp", 2), ("mchp", 2), ("fstream", 2), ("rdp", 1),
                ("lnr", 1), ("tmp", 2),
            ]
            pools = {nm: stack.enter_context(tc.tile_pool(name=nm, bufs=bu))
                     for nm, bu in pool_specs}
            (small, wres, kinp, qinp, qfp, qtp, ktc, vp, expp, ctxp, xp,
             hp, x2p, outp, sqp, mchp, fstream, rdp, lnr, tmpp) = (
                pools[nm] for nm, _ in pool_specs)
            # ---- persistent small tiles ----
            ones_col = small.tile([P, 1], f32r, tag="ones_col")
            nc.sync.dma_start(ones_col[:], ones_col_d.ap())
            ones_row = small.tile([1, T], f32r, tag="ones_row")
            nc.sync.dma_start(ones_row[:], ones_row_d.ap())
            ones_row_bf = small.tile([1, T], bf16, tag="ones_row_bf")
            nc.vector.tensor_copy(ones_row_bf[:], F(ones_row[:]))
            o64f = small.tile([1, HD], f32, tag="ones64f")
            nc.vector.memset(o64f[:], 1.0)
            ones64_bf = small.tile([1, HD], bf16, tag="ones64")
            nc.vector.tensor_copy(ones64_bf[:], o64f[:])
            onesc_f = small.tile([P, 1], f32, tag="onesc_f")
            nc.vector.memset(onesc_f[:], 1.0)
            eps_t = small.tile([1, 1], f32, tag="eps")
            nc.vector.memset(eps_t[:], EPS)

            ln2gb = small.tile([2, D], bf16, tag="ln2gb")
            nc.sync.dma_start(ln2gb[:], lnrows_d.ap()[0:2, :])
            lnpgb = small.tile([2, D], bf16, tag="lnpgb")
            nc.sync.dma_start(lnpgb[:], lnrows_d.ap()[2:4, :])

            bq_pc = small.tile([P, DC], f32, tag="bq_pc")
            nc.sync.dma_start(bq_pc[:], bq_d.ap())
            bk_pc = small.tile([P, DC], f32, tag="bk_pc")
            nc.sync.dma_start(bk_pc[:], bk_d.ap())
            bo_pc = small.tile([P, DC], f32, tag="bo_pc")
            nc.sync.dma_start(bo_pc[:], bo_d.ap())
            f1b_pc = small.tile([P, FFC], f32, tag="f1b_pc")
            nc.sync.dma_start(f1b_pc[:], f1b_d.ap())
            f2b_pc = small.tile([P, DC], f32, tag="f2b_pc")
            nc.sync.dma_start(f2b_pc[:], f2b_d.ap())

            bv_row = None
            if use_bv:
                bv_row = small.tile([1, D], f32r, tag="bv_row")
                nc.sync.dma_start(bv_row[:], bv_d.ap())

            # ---- resident fp8 attention weights ----
            w8 = {}
            for nm, dram in (("q", w8q_d), ("k", w8k_d), ("v", w8v_d),
                             ("o", w8o_d)):
                t8 = wres.tile([P, KP, 2, D], fp8, tag=f"w8{nm}")
                nc.gpsimd.dma_start(t8[:], dram.ap())
                w8[nm] = t8

            # ---- per-batch persistent activations ----
            kin, qin8, qinf, qt, v8, ctx8, xT, hT, x2T = \
                {}, {}, {}, {}, {}, {}, {}, {}, {}

            def attn(b, psBIG, psSC, psCTX, psBC):
                kin[b] = kinp.tile([P, KP, 2, S], fp8, tag="kin", name=f"kin{b}")
                nc.gpsimd.dma_start(kin[b][:], kT8_d.ap()[b])
                qin8[b] = qinp.tile([P, KP, 2, T], fp8, tag="qin8", name=f"qin8_{b}")
                nc.sync.dma_start(qin8[b][:], qT8_d.ap()[b])

                # ---- Q projection (fp8 DR) -> qt bf16 [128, 6, 2, T]
                # head-h slice lives in its 64 rows, other 64 rows zero,
                # so scores contract K=128 against the full ktch chunk.
                qt[b] = qtp.tile([P, DC, 2, T], bf16, tag="qt", name=f"qt{b}")
                nc.vector.memset(qt[b][0:HD, :, 1, :], 0.0)
                nc.vector.memset(qt[b][HD:P, :, 0, :], 0.0)
                for mo in range(DC):
                    ps = psBIG.tile([P, T], f32, tag="psBIG")
                    for kp in range(KP):
                        nc.tensor.matmul(
                            ps[:], w8["q"][:, kp, :, mo * P:(mo + 1) * P],
                            qin8[b][:, kp, :, :],
                            start=(kp == 0), stop=(kp == KP - 1), perf_mode=DR)
                    nc.vector.tensor_scalar_add(
                        qt[b][0:HD, mo, 0, :], ps[0:HD, :],
                        bq_pc[0:HD, mo:mo + 1])
                    nc.vector.tensor_scalar_add(
                        qt[b][HD:P, mo, 1, :], ps[HD:P, :],
                        bq_pc[HD:P, mo:mo + 1])

                ktchs = {}

                def kproj_pre(c):
                    ktch = ktc.tile([P, S], bf16, tag="ktc", name=f"ktc{c}")
                    for st in range(4):
                        ps = psBIG.tile([P, T], f32, tag="psBIG")
                        for kp in range(KP):
                            nc.tensor.matmul(
                                ps[:], w8["k"][:, kp, :, c * P:(c + 1) * P],
                                kin[b][:, kp, :, st * T:(st + 1) * T],
                                start=(kp == 0), stop=(kp == KP - 1),
                                perf_mode=DR)
                        nc.vector.tensor_scalar_add(
                            ktch[:, st * T:(st + 1) * T], ps[:],
                            bk_pc[:, c:c + 1])
                    return ktch

                # ---- V projection (fp8 DR) -> v8 [128, 8, 2, 12, 65] ----
                v8[b] = vp.tile([P, SP, 2, H, HD + 4], fp8, tag="v8", name=f"v8_{b}")
                vflat = v8[b][:].rearrange("p a b h e -> p (a b h) e")
                nc.vector.memset(vflat[:, :, HD + 1:HD + 4], 0.0)
                nc.vector.tensor_copy(
                    vflat[:, :, HD:HD + 1],
                    onesc_f[:].to_broadcast([P, SP * 2 * H, 1]))
                bv_bc = None
                if use_bv:
                    bv_bc = small.tile([P, D], f32, tag="bv_bc")
                    for half in range(2):
                        ps = psBIG.tile([P, T], f32, tag="psBIG")
                        nc.tensor.matmul(
                            ps[:, 0:384], ones_row[:, 0:P],
                            bv_row[:, half * 384:(half + 1) * 384],
                            start=True, stop=True)
                        nc.vector.tensor_copy(
                            bv_bc[:, half * 384:(half + 1) * 384], ps[:, 0:384])
                for so in range(SC):
                    if so == 0:
                        ktchs[0] = kproj_pre(0)
                    if so == 8:
                        ktchs[1] = kproj_pre(1)
                    for half in range(2):
                        ps = psBIG.tile([P, T], f32, tag="psBIG")
                        for kp in range(KP):
                            nc.tensor.matmul(
                                ps[:, 0:384],
                                kin[b][:, kp, :, so * P:(so + 1) * P],
                                w8["v"][:, kp, :, half * 384:(half + 1) * 384],
                                start=(kp == 0), stop=(kp == KP - 1),
                                perf_mode=DR)
                        dstv = v8[b][:, so // 2, so % 2,
                                     6 * half:6 * half + 6, 0:HD]
                        if use_bv:
                            nc.vector.tensor_tensor(
                                dstv, ps[:, 0:384],
                                bv_bc[:, half * 384:(half + 1) * 384], ALU.add)
                        else:
                            nc.vector.tensor_copy(dstv, ps[:, 0:384])

                # ---- attention per feature-chunk (2 heads) ----
                ctx8[b] = ctxp.tile([P, KP, 2, T], fp8, tag="ctx8", name=f"ctx8_{b}")
                for c in range(DC):
                    ktch = ktchs.pop(c) if c in ktchs else kproj_pre(c)
                    if c + 1 < DC and c >= 1:
                        ktchs[c + 1] = kproj_pre(c + 1)

                    ps_ctx = [psCTX.tile([P, T], f32, tag="psCTX",
                                         name=f"ps_ctx{i}") for i in range(2)]
                    for sp in range(SP):
                        pscs = [psSC.tile([P, 2, T], f32, tag="psSC",
                                          name=f"ps_sc{i}") for i in range(2)]
                        for j in range(2):
                            so = sp * 2 + j
                            for hh in range(2):
                                nc.tensor.matmul(
                                    pscs[hh][:, j, :],
                                    ktch[:, so * P:(so + 1) * P],
                                    qt[b][:, c, hh, :],
                                    start=True, stop=True)
                        exs = []
                        for hh in range(2):
                            ex = expp.tile([P, 2, T], fp8, tag="exp",
                                           name=f"ex{hh}")
                            nc.scalar.activation(ex[:], pscs[hh][:], AF.Exp)
                            exs.append(ex)
                        for hh in range(2):
                            h = 2 * c + hh
                            for j in range(2):
                                nc.tensor.matmul(
                                    ps_ctx[hh][0:HD + 4, :],
                                    v8[b][:, sp, j, h, :],
                                    exs[hh][:, j, :],
                                    start=(sp == 0 and j == 0),
                                    stop=(sp == SP - 1 and j == 1))
                    for hh in range(2):
                        h = 2 * c + hh
                        den_sb = rdp.tile([1, T], f32, tag="den_sb")
                        nc.vector.tensor_copy(den_sb[:],
                                              ps_ctx[hh][HD:HD + 1, :])
                        rden = rdp.tile([1, T], f32, tag="rden")
                        nc.vector.reciprocal_approx_fast(
                            out=rden[:], in_=den_sb[:])
                        rden_bf = rdp.tile([1, T], bf16, tag="rden_bf")
                        nc.vector.tensor_copy(rden_bf[:], rden[:])
                        ps_bc = psBC.tile([HD, T], f32, tag="psBC")
                        nc.tensor.matmul(ps_bc[:], ones64_bf[:],
                                         rden_bf[:], start=True, stop=True)
                        bc_sb = tmpp.tile([HD, T], bf16, tag="bc_sb")
                        nc.vector.tensor_copy(bc_sb[:], ps_bc[:])
                        nc.vector.tensor_tensor(
                            ctx8[b][(h % 2) * HD:(h % 2) * HD + HD,
                                    (h // 2) // 2, (h // 2) % 2, :],
                            ps_ctx[hh][0:HD, :], bc_sb[:], ALU.mult)

                # ---- out projection (fp8 DR) + residual -> xT f32r ----
                xT[b] = xp.tile([P, DC, T], f32r, tag="xT", name=f"xT{b}")
                for mo in range(DC):
                    qf = qfp.tile([P, T], f32r, tag="qinf")
                    nc.sync.dma_start(qf[:], qTf_d.ap()[b][:, mo, :])
                    ps = psBIG.tile([P, T], f32, tag="psBIG")
                    for kp in range(KP):
                        nc.tensor.matmul(
                            ps[:], w8["o"][:, kp, :, mo * P:(mo + 1) * P],
                            ctx8[b][:, kp, :, :],
                            start=(kp == 0), stop=(kp == KP - 1), perf_mode=DR)
                    nc.vector.scalar_tensor_tensor(
                        xT[b][:, mo, :], ps[:], bo_pc[:, mo:mo + 1],
                        F(qf[:]), op0=ALU.add, op1=ALU.add)

            def ln_pass(xsrc, gb_pair, ps_st, ps_bc, dst_alloc):
                """LayerNorm over the partition(feature) dim.
                xsrc [P, DC, T] f32r; dst_alloc(c2) -> (dst_ap, finish|None)."""
                psum_mu = ps_st.tile([1, T], f32, tag="st_mu")
                psum_sq = ps_st.tile([1, T], f32, tag="st_sq")
                for c2 in range(DC):
                    nc.tensor.matmul(psum_mu[:], ones_col[:], xsrc[:, c2, :],
                                     start=(c2 == 0), stop=(c2 == DC - 1))
                sqt = []
                for c2 in range(DC):
                    sq = sqp.tile([P, T], f32r, tag="lnsq")
                    nc.vector.tensor_mul(sq[:], F(xsrc[:, c2, :]),
                                         F(xsrc[:, c2, :]))
                    sqt.append(sq)
                for c2 in range(DC):
                    nc.tensor.matmul(psum_sq[:], ones_col[:], sqt[c2][:],
                                     start=(c2 == 0), stop=(c2 == DC - 1))
                mu_t = lnr.tile([1, T], f32, tag="lnmu")
                mu2_t = lnr.tile([1, T], f32, tag="lnmu2")
                rs_t = lnr.tile([1, T], bf16, tag="lnrs")
                mrs_t = lnr.tile([1, T], bf16, tag="lnmrs")
                mu_f, mu2_f = mu_t[:], mu2_t[:]
                rs_f, mrs_f = rs_t[:], mrs_t[:]
                nc.vector.tensor_scalar_mul(mu_f, psum_mu[:], 1.0 / D)
                nc.vector.tensor_tensor(mu2_f, mu_f, mu_f, ALU.mult)
                var_f = mu2_f
                nc.vector.scalar_tensor_tensor(
                    var_f, psum_sq[:], 1.0 / D, mu2_f,
                    op0=ALU.mult, op1=ALU.subtract)
                nc.scalar.activation(rs_f, var_f, AF.Abs_reciprocal_sqrt,
                                     bias=eps_t[:])
                nc.vector.tensor_tensor(mrs_f, mu_f, rs_f, ALU.mult)
                m1 = lnr.tile([2, T], bf16, tag="lnm1")
                nc.vector.tensor_copy(m1[0:1, :], mrs_f)
                nc.sync.dma_start(m1[1:2, :], ones_row_bf[:])
                for c2 in range(DC):
                    bcA = ps_bc.tile([P, T], f32, tag="ln_bcA")
                    bcB = ps_bc.tile([P, T], f32, tag="ln_bcB")
                    gsl = gb_pair[0:1, c2 * P:(c2 + 1) * P]
                    gbsl = gb_pair[:, c2 * P:(c2 + 1) * P]
                    nc.tensor.matmul(bcA[:], gsl, rs_f,
                                     start=True, stop=True)
                    nc.tensor.matmul(bcB[:], gbsl, m1[:],
                                     start=True, stop=True)
                    dst, finish = dst_alloc(c2)
                    tmp = tmpp.tile([P, T], f32, tag="ln_tmp")
                    nc.vector.tensor_tensor(tmp[:], F(xsrc[:, c2, :]), bcA[:],
                                            ALU.mult)
                    nc.vector.tensor_tensor(dst, tmp[:], bcB[:], ALU.subtract)
                    if finish is not None:
                        finish()

            def mlp(b, psF1, psF2):
                x2T[b] = x2p.tile([P, DC, T], f32r, tag="x2T", name=f"x2T{b}")
                ps_f2 = [psF2.tile([P, T], f32, tag="psF2", name=f"ps_f2_{i}")
                         for i in range(DC)]
                if mlp_fp8:
                    hview = hT[b][:]
                    mch = None
                    for fo in range(FFC):
                        f1_sl = fstream.tile([P, KP, 2, P], fp8, tag="f1_sl")
                        nc.sync.dma_start(f1_sl[:], fc1_d.ap()[fo])
                        if fo % 2 == 0:
                            f2_sl = fstream.tile([P, 2, D], fp8, tag="f2_sl")
                            nc.sync.dma_start(f2_sl[:], fc2_d.ap()[fo // 2])
                            mch = mchp.tile([P, 2, T], fp8, tag="mch")
                        ps1 = psF1.tile([P, T], f32, tag="psF1")
                        for kp in range(KP):
                            nc.tensor.matmul(
                                ps1[:], f1_sl[:, kp, :, :], hview[:, kp, :, :],
                                start=(kp == 0), stop=(kp == KP - 1),
                                perf_mode=DR)
                        nc.scalar.activation(mch[:, fo % 2, :], ps1[:],
                                             AF.Gelu, bias=f1b_pc[:, fo:fo + 1])
                        if fo % 2 == 1:
                            for mo in range(DC):
                                nc.tensor.matmul(
                                    ps_f2[mo][:],
                                    f2_sl[:, :, mo * P:(mo + 1) * P], mch[:],
                                    start=(fo == 1), stop=(fo == FFC - 1),
                                    perf_mode=DR)
                else:
                    for fo in range(FFC):
                        f1_sl = fstream.tile([P, DC, P], bf16, tag="f1_sl")
                        nc.sync.dma_start(f1_sl[:], fc1_d.ap()[fo])
                        f2_sl = fstream.tile([P, D], bf16, tag="f2_sl")
                        nc.sync.dma_start(f2_sl[:], fc2_d.ap()[fo])
                        ps1 = psF1.tile([P, T], f32, tag="psF1")
                        for ki in range(DC):
                            nc.tensor.matmul(ps1[:], f1_sl[:, ki, :],
                                             hT[b][:, ki, :],
                                             start=(ki == 0),
                                             stop=(ki == DC - 1))
                        mch = mchp.tile([P, T], bf16, tag="mch")
                        nc.scalar.activation(mch[:], ps1[:], AF.Gelu,
                                             bias=f1b_pc[:, fo:fo + 1])
                        for mo in range(DC):
                            nc.tensor.matmul(
                                ps_f2[mo][:], f2_sl[:, mo * P:(mo + 1) * P],
                                mch[:],
                                start=(fo == 0), stop=(fo == FFC - 1))
                for mo in range(DC):
                    nc.vector.scalar_tensor_tensor(
                        x2T[b][:, mo, :], ps_f2[mo][:], f2b_pc[:, mo:mo + 1],
                        F(xT[b][:, mo, :]), op0=ALU.add, op1=ALU.add)

            # ================= schedule =================
            with (
                tc.tile_pool(name="psBIG", bufs=1, space="PSUM") as psBIG,
                tc.tile_pool(name="psSC", bufs=2, space="PSUM") as psSC,
                tc.tile_pool(name="psCTX", bufs=2, space="PSUM") as psCTX,
                tc.tile_pool(name="psBC", bufs=1, space="PSUM") as psBC,
            ):
                for b in range(BPC):
                    attn(b, psBIG, psSC, psCTX, psBC)

            ln2g, ln2bn = ln2gb[:], ln2gb[:]
            lnpg, lnpbn = lnpgb[:], lnpgb[:]

            with (
                tc.tile_pool(name="psST", bufs=1, space="PSUM") as psST,
                tc.tile_pool(name="psLB", bufs=2, space="PSUM") as psLB,
            ):
                for b in range(BPC):
                    if mlp_fp8:
                        hT[b] = hp.tile([P, KP, 2, T], fp8, tag="hT", name=f"hT{b}")
                        hview = hT[b][:].rearrange("p a b t -> p (a b) t")
                    else:
                        hT[b] = hp.tile([P, DC, T], bf16, tag="hT", name=f"hT{b}")
                        hview = hT[b][:]
                    ln_pass(xT[b], ln2g, psST, psLB,
                            lambda c2, hv=hview: (hv[:, c2, :], None))

            with (
                tc.tile_pool(name="psF1", bufs=2, space="PSUM") as psF1,
                tc.tile_pool(name="psF2", bufs=6, space="PSUM") as psF2,
            ):
                for b in range(BPC):
                    mlp(b, psF1, psF2)

            with (
                tc.tile_pool(name="psST2", bufs=1, space="PSUM") as psST2,
                tc.tile_pool(name="psLB2", bufs=2, space="PSUM") as psLB2,
            ):
                for b in range(BPC):
                    def out_alloc(c2, b=b):
                        t = outp.tile([P, T], f32, tag="outT")
                        fin = (lambda t=t, c2=c2, b=b:
                               nc.sync.dma_start(out_d.ap()[b][:, c2, :], t[:]))
                        return t[:], fin
                    ln_pass(x2T[b], lnpg, psST2, psLB2, out_alloc)

    nc.compile()
    return nc


def _get_nc(use_bv: bool, mlp_fp8: bool):
    key = ("nc", use_bv, mlp_fp8)
    if key not in _cached:
        _cached[key] = _build(use_bv, mlp_fp8)
    return _cached[key]


def _to_fp8(x):
    return np.asarray(x, np.float32).astype(ml_dtypes.float8_e4m3)


def _to_bf16(x):
    return np.asarray(x, np.float32).astype(ml_dtypes.bfloat16)


def _tile_kp(wT):
    """[d_in, n] -> [P, KP, 2, n] with d_in = (kp*2 + i)*P + p."""
    n = wT.shape[1]
    return np.ascontiguousarray(wT.reshape(KP, 2, P, n).transpose(2, 0, 1, 3))


def _col_pc(v, nch):
    """[n] -> [P, nch] with n = c*P + p."""
    return np.ascontiguousarray(np.asarray(v, np.float32).reshape(nch, P).T)


def _prep_shared(wq, bq, wk, bk, wv, bv, wo, bo,
                 ln2_g, ln2_b, fc1_w, fc1_b, fc2_w, fc2_b, lnp_g, lnp_b,
                 mlp_fp8):
    f = np.float32
    c = np.ascontiguousarray
    sc = np.float32(SCALE)
    lnrows = np.stack([
        np.asarray(ln2_g, f), -np.asarray(ln2_b, f),
        np.asarray(lnp_g, f), -np.asarray(lnp_b, f)])
    shared = {
        "w8q": _to_fp8(_tile_kp(np.asarray(wq, f).T * sc)),
        "w8k": _to_fp8(_tile_kp(np.asarray(wk, f).T)),
        "w8v": _to_fp8(_tile_kp(np.asarray(wv, f).T)),
        "w8o": _to_fp8(_tile_kp(np.asarray(wo, f).T)),
        "bqv": _col_pc(np.asarray(bq, f) * sc, DC),
        "bkv": _col_pc(bk, DC),
        "bov": _col_pc(bo, DC),
        "bvv": c(np.asarray(bv, f).reshape(1, D)),
        "f1b": _col_pc(fc1_b, FFC),
        "f2b": _col_pc(fc2_b, DC),
        "lnrows": _to_bf16(lnrows),
        "ones_col": np.ones((P, 1), f),
        "ones_row": np.ones((1, T), f),
    }
    f1T = np.asarray(fc1_w, f).T           # [D, FF]
    f2T = np.asarray(fc2_w, f).T           # [FF, D]
    if mlp_fp8:
        # fc1: [FFC, P, KP, 2, P]; fc2: [FFC//2, P, 2, D] (ff = fo*P + p)
        shared["fc1_t"] = _to_fp8(
            f1T.reshape(KP, 2, P, FFC, P).transpose(3, 2, 0, 1, 4))
        shared["fc2_t"] = _to_fp8(
            f2T.reshape(FFC // 2, 2, P, D).transpose(0, 2, 1, 3))
    else:
        shared["fc1_t"] = _to_bf16(
            f1T.reshape(DC, P, FFC, P).transpose(2, 1, 0, 3))
        shared["fc2_t"] = _to_bf16(f2T.reshape(FFC, P, D))
    return shared


def _prep_batch(query_b, key_b):
    """Per-batch tensors: query_b [T, D], key_b [S, D]."""
    f = np.float32
    qT = np.asarray(query_b, f).T          # [D, T]
    kT = np.asarray(key_b, f).T            # [D, S]
    return (
        _to_fp8(qT.reshape(KP, 2, P, T).transpose(2, 0, 1, 3)),
        np.ascontiguousarray(qT.reshape(DC, P, T).transpose(1, 0, 2)),
        _to_fp8(kT.reshape(KP, 2, P, S).transpose(2, 0, 1, 3)),
    )


def kernel(query, key, wq, bq, wk, bk, wv, bv, wo, bo,
           ln2_g, ln2_b, fc1_w, fc1_b, fc2_w, fc2_b, lnp_g, lnp_b):
    from concourse.bass_utils import run_bass_kernel_spmd

    query = np.asarray(query, np.float32)
    key = np.asarray(key, np.float32)
    use_bv = bool(np.any(np.asarray(bv)))
    nc = _get_nc(use_bv, MLP_FP8)

    shared = _prep_shared(wq, bq, wk, bk, wv, bv, wo, bo,
                          ln2_g, ln2_b, fc1_w, fc1_b, fc2_w, fc2_b,
                          lnp_g, lnp_b, MLP_FP8)
    in_maps = []
    for core in range(NCORES):
        m = dict(shared)
        q8s, qfs, k8s = [], [], []
        for j in range(BPC):
            b = core * BPC + j
            q8, qf, k8 = _prep_batch(query[b], key[b])
            q8s.append(q8)
            qfs.append(qf)
            k8s.append(k8)
        m["qT8"] = np.stack(q8s)
        m["qTf"] = np.stack(qfs)
        m["kT8"] = np.stack(k8s)
        in_maps.append(m)

    res = run_bass_kernel_spmd(nc, in_maps, core_ids=list(range(NCORES)))
    kernel._last_result = res
    out = np.stack([r["out"] for r in res.results])   # [NC, BPC, P, DC, T]
    # [core, b, p, c, t] -> [B, T, c*P+p]
    out = out.reshape(B, P, DC, T).transpose(0, 3, 2, 1).reshape(B, T, D)
    return np.ascontiguousarray(out)
